# revision 1
# baseline (speedup 1.0000x reference)
"""Expert-parallel MoE MLP (BaseMLPExperts) for 8 TRN2 NeuronCores.

Reference computation (per expert e):
    y[:, e, :] = gelu_exact(x[:, e, :] @ wi[e]) @ wo[e]
with T=8192 tokens, E=8 experts, H=1024 hidden, I=4096 intermediate, fp32.

Sharding: expert-parallel — core e owns expert e (its x slice, wi[e], wo[e]).
No cross-core communication.

Per-core device kernel (all matmuls in f32r = TF32-on-PE at full PE rate,
fp32 PSUM accumulation; measured rel-err ~2e-4 end to end):
  Phase 1: h1T[I, T] = gelu(x @ wi) transposed, streamed by 512-token tiles;
           wi SBUF-resident (128KB/partition, split into lo/hi halves so the
           last token tile can release them in stages); GELU (exact erf form)
           applied on PSUM eviction by the ACT engine, written to DRAM
           scratch as f32r.
  Phase 2: y[T, H] = h1 @ wo, streamed by 128-token blocks; wo SBUF-resident
           (prefetched in three pieces: 8 i-tiles during phase 1, 16 as
           wi_hi's space frees, 8 as wi_lo's space frees); h1T tiles act as
           the stationary matmul operand so y comes out untransposed.

DMA issue queues (SP/GpSimd/ACT sequencers) are spread and emission-ordered
so the first matmul group only waits for ~4MB of priming traffic.

Host side: transposes x slices to xT (H-major), shards, runs the SPMD kernel
on cores 0-7, stacks per-core y into [T, E, H].
"""

import numpy as np

import concourse.bass as bass
import concourse.mybir as mybir
import concourse.tile as tile
from concourse import bacc
from concourse.bass_utils import run_bass_kernel_spmd

T, E, H, I = 8192, 8, 1024, 4096
P = 128
F32 = mybir.dt.float32
F32R = mybir.dt.float32r

TT1 = 512            # phase-1 token tile
NT1 = T // TT1       # 16
HT = H // P          # 8 k-tiles for GEMM1
IT = I // P          # 32 i-tiles
TT2 = 128            # phase-2 token block
NT2 = T // TT2       # 64

# run_bass_kernel_spmd kwargs injected by test harness (e.g. trace=True)
RUN_KWARGS: dict = {}
LAST_RESULT = None

_NC = None


def _build():
    nc = bacc.Bacc("TRN2", target_bir_lowering=False, debug=False, num_devices=8)

    xT = nc.dram_tensor("xT", [H, T], F32R, kind="ExternalInput").ap()
    wi = nc.dram_tensor("wi", [H, I], F32R, kind="ExternalInput").ap()
    wo = nc.dram_tensor("wo", [I, H], F32R, kind="ExternalInput").ap()
    y = nc.dram_tensor("y", [T, H], F32, kind="ExternalOutput").ap()

    xT_r = xT.rearrange("(ho p) t -> p ho t", p=P)      # [128, 8, T]
    wi_r = wi.rearrange("(ho p) i -> p ho i", p=P)      # [128, 8, I]
    wo_r = wo.rearrange("(io p) h -> p io h", p=P)      # [128, 32, H]

    with tile.TileContext(nc) as tc:
        with tc.tile_pool(name="h1dram", bufs=1, space="DRAM") as dpool:
            # h1T scratch: one [I, TT1] block per phase-1 token tile
            h1b = [
                dpool.tile([I, TT1], F32R, name=f"h1b{t}", tag=f"h1b{t}")
                for t in range(NT1)
            ]

            # wi lives in two 64KB/partition tiles; the last token tile
            # consumes wi chunk by chunk (512 i-columns each), and each
            # freed 16KB chunk space is immediately refilled with a 4-i-tile
            # piece of wo via matching 4D APs (Tile's subtile tracker
            # serializes the WAR), so wo streams in under phase-1's tail.
            wo_pre_pool = tc.alloc_tile_pool(name="wo_pre_pool", bufs=1)
            wi_pool = tc.alloc_tile_pool(name="wi_pool", bufs=1)
            wo_pre = wo_pre_pool.tile([P, 8, H], F32R, name="wo_pre")
            wi_lo = wi_pool.tile([P, HT, I // 2], F32R, name="wi_lo")
            wi_hi = wi_pool.tile([P, HT, I // 2], F32R, name="wi_hi")

            def wi_slice(h, i):
                if i < 16:
                    return wi_lo[:, h, i * P : (i + 1) * P]
                return wi_hi[:, h, (i - 16) * P : (i - 15) * P]

            def wi_chunk_space(q):
                # 16KB/partition column range of wi chunk q (i-cols q*512..)
                t = wi_lo if q < 4 else wi_hi
                return t[:, :, (q % 4) * 512 : (q % 4 + 1) * 512]

            # tt15 processes wi chunks in this order: chunks 2,3 first (their
            # spaces host the h1 blocks of the first two phase-2 token
            # blocks, which must be resident BEFORE phase 1 ends), then the
            # wo hosts in phase-2 consumption order (wo piece k = i-tiles
            # 8+4k..11+4k lands in chunk WO_DEST[k]).
            TT15_ORDER = [2, 3, 4, 5, 6, 7, 0, 1]
            WO_DEST = [4, 5, 6, 7, 0, 1]
            H1I_DEST = [2, 3]

            def load_wo_piece(k, eng0, eng1):
                i0 = 8 + 4 * k
                dst4 = wi_chunk_space(WO_DEST[k]).rearrange(
                    "p (a s) c -> p s a c", s=2
                )
                for s, eng in ((0, eng0), (1, eng1)):
                    eng.dma_start(
                        out=dst4[:, s],
                        in_=wo_r[:, i0 : i0 + 4, s * 512 : (s + 1) * 512],
                    )

            def wo_slice(i, hh):
                if i < 8:
                    return wo_pre[:, i, hh * 512 : (hh + 1) * 512]
                k, j = (i - 8) // 4, (i - 8) % 4
                return wi_chunk_space(WO_DEST[k])[:, 2 * j + hh, :]

            def load_h1i_alias(tb, eng):
                # h1 block of token block tb -> wi chunk space H1I_DEST[tb]
                tt, tsub = tb // 4, tb % 4
                src = h1b[tt].rearrange("(io p) t -> p io t", p=P)
                cs = wi_chunk_space(H1I_DEST[tb])
                for h in range(HT):
                    eng.dma_start(
                        out=cs[:, h, :].rearrange("p (a b) -> p a b", b=TT2),
                        in_=src[:, 4 * h : 4 * h + 4, tsub * TT2 : (tsub + 1) * TT2],
                    )

            def h1i_alias_slice(tb, j):
                cs = wi_chunk_space(H1I_DEST[tb])
                return cs[:, j // 4, (j % 4) * P : (j % 4 + 1) * P]

            with (
                tc.tile_pool(name="xt_pool", bufs=2) as xt_pool,
                tc.tile_pool(name="h1o_pool", bufs=6) as h1o_pool,
                tc.tile_pool(name="ps1_pool", bufs=8, space="PSUM") as ps1_pool,
            ):
                def load_xt(tt):
                    t0 = tt * TT1
                    xt = xt_pool.tile([P, HT, TT1], F32R, name="xt", tag="xt")
                    for g, eng in ((0, nc.sync), (1, nc.scalar)):
                        eng.dma_start(
                            out=xt[:, 4 * g : 4 * g + 4, :],
                            in_=xT_r[:, 4 * g : 4 * g + 4, t0 : t0 + TT1],
                        )
                    return xt

                # Priming: xt(tt=0) then the wi chunks in consumption order,
                # alternating ACT/SP queues. GpSimd carries ONLY the h1
                # stores in phase 1 — mixing big preloads onto it delays
                # store completions, which stalls gelu via h1o-slot WAR.
                # A single dma_start lands on one DMA engine (~90-150GB/s),
                # so the first-matmul critical set (xt0+xt1 + wi chunk 0) is
                # split into small pieces spread across queues/engines.
                def load_xt_split(tt):
                    t0 = tt * TT1
                    xt = xt_pool.tile([P, HT, TT1], F32R, name="xt", tag="xt")
                    for q, eng in ((0, nc.sync), (1, nc.scalar),
                                   (2, nc.sync), (3, nc.scalar)):
                        eng.dma_start(
                            out=xt[:, 2 * q : 2 * q + 2, :],
                            in_=xT_r[:, 2 * q : 2 * q + 2, t0 : t0 + TT1],
                        )
                    return xt

                xt0 = load_xt_split(0)
                for half in range(2):  # chunk 0, both halves on idle GpSimd
                    nc.gpsimd.dma_start(
                        out=wi_lo[:, 4 * half : 4 * half + 4, 0:512],
                        in_=wi_r[:, 4 * half : 4 * half + 4, 0:512],
                    )
                xt1 = load_xt_split(1)
                # chunks 1..7 as h-halves alternating SP/ACT (GpSimd must be
                # clear before the h1 stores start)
                engs2 = [nc.sync, nc.scalar]
                n = 0
                for g in range(1, 8):
                    t = wi_lo if g < 4 else wi_hi
                    cc = (g % 4) * 512
                    for half in range(2):
                        engs2[n % 2].dma_start(
                            out=t[:, 4 * half : 4 * half + 4, cc : cc + 512],
                            in_=wi_r[:, 4 * half : 4 * half + 4,
                                     g * 512 : (g + 1) * 512],
                        )
                        n += 1

                def igroup(tt, i, xt):
                    ps = ps1_pool.tile([P, TT1], F32, name="ps1", tag="ps1")
                    for h in range(HT):
                        nc.tensor.matmul(
                            ps[:],
                            wi_slice(h, i),
                            xt[:, h, :],
                            start=(h == 0),
                            stop=(h == HT - 1),
                        )
                    h1o = h1o_pool.tile([P, TT1], F32R, name="h1o", tag="h1o")
                    nc.scalar.activation(
                        h1o[:], ps[:], mybir.ActivationFunctionType.Gelu
                    )
                    nc.gpsimd.dma_start(
                        out=h1b[tt][i * P : (i + 1) * P, :], in_=h1o[:]
                    )

                # Token tiles 0 and 1 run interleaved chunk-major over the
                # first 4 wi chunks, halving the wi consumption rate while
                # the priming DMA burst streams in; their tails then run
                # tile-major so xt(2) can prefetch into tile 0's slot.
                for c in range(4):
                    for tt, xt in ((0, xt0), (1, xt1)):
                        for i in range(4 * c, 4 * c + 4):
                            igroup(tt, i, xt)
                for tt, xt in ((0, xt0), (1, xt1)):
                    if tt == 1:
                        xt_cur = load_xt(2)
                    for i in range(16, IT):
                        igroup(tt, i, xt)

                for tt in range(2, NT1 - 1):
                    xt_nxt = load_xt(tt + 1)
                    for i in range(IT):
                        igroup(tt, i, xt_cur)
                    if tt == 3:
                        # wo_pre loads once the priming burst has drained
                        for g, eng in ((0, nc.sync), (1, nc.scalar)):
                            eng.dma_start(
                                out=wo_pre[:, 4 * g : 4 * g + 4, :],
                                in_=wo_r[:, 4 * g : 4 * g + 4, :],
                            )
                    xt_cur = xt_nxt

                # Last token tile: consume wi chunk by chunk; right after a
                # chunk's last read, stream the phase-2 h1 prefetch (SP) or
                # the matching wo piece (SP/ACT; GpSimd still owns the h1
                # stores) into its space.
                for n, q in enumerate(TT15_ORDER):
                    for i in range(4 * q, 4 * q + 4):
                        igroup(NT1 - 1, i, xt_cur)
                    if n < 2:
                        load_h1i_alias(n, nc.sync)
                    else:
                        load_wo_piece(n - 2, nc.scalar, nc.sync)

            # ---------------- Phase 2: y = h1 @ wo ----------------------
            with (
                tc.tile_pool(name="h1i_pool", bufs=2) as h1i_pool,
                tc.tile_pool(name="yo_pool", bufs=3) as yo_pool,
                tc.tile_pool(name="ps2_pool", bufs=8, space="PSUM") as ps2_pool,
            ):
                def load_h1i(tb):
                    tt, tsub = tb // 4, tb % 4
                    src = h1b[tt].rearrange("(io p) t -> p io t", p=P)
                    h1i = h1i_pool.tile([P, IT, TT2], F32R, name="h1i", tag="h1i")
                    for g in range(4):
                        eng = nc.sync if g % 2 == 0 else nc.gpsimd
                        eng.dma_start(
                            out=h1i[:, 8 * g : 8 * g + 8, :],
                            in_=src[
                                :, 8 * g : 8 * g + 8, tsub * TT2 : (tsub + 1) * TT2
                            ],
                        )
                    return h1i

                pending = [load_h1i(2), load_h1i(3)]
                for tb in range(NT2):
                    if tb < 2:
                        h1sl = lambda j: h1i_alias_slice(tb, j)  # noqa: B023
                    else:
                        h1i = pending.pop(0)
                        h1sl = lambda j: h1i[:, j, :]  # noqa: B023
                    yo = yo_pool.tile([P, H], F32, name="yo", tag="yo")
                    # i outer / h-half inner: each stationary h1 tile feeds
                    # two matmuls back to back (halves the weight-load duty)
                    pss = [
                        ps2_pool.tile([P, 512], F32, name="ps2", tag="ps2")
                        for _ in range(2)
                    ]
                    for i in range(IT):
                        for hh in range(2):
                            nc.tensor.matmul(
                                pss[hh][:],
                                h1sl(i),
                                wo_slice(i, hh),
                                start=(i == 0),
                                stop=(i == IT - 1),
                            )
                    for hh in range(2):
                        nc.vector.tensor_copy(
                            yo[:, hh * 512 : (hh + 1) * 512], pss[hh][:]
                        )
                    nc.scalar.dma_start(
                        out=y[tb * TT2 : (tb + 1) * TT2, :], in_=yo[:]
                    )
                    if tb + 4 < NT2:
                        pending.append(load_h1i(tb + 4))
            wi_pool.release()
            wo_pre_pool.release()

    nc.compile()
    return nc


def kernel(x: np.ndarray, wi: np.ndarray, wo: np.ndarray) -> np.ndarray:
    global _NC, LAST_RESULT
    x = np.asarray(x, dtype=np.float32)
    wi = np.asarray(wi, dtype=np.float32)
    wo = np.asarray(wo, dtype=np.float32)
    assert x.shape == (T, E, H) and wi.shape == (E, H, I) and wo.shape == (E, I, H)

    if _NC is None:
        _NC = _build()

    in_maps = [
        {
            "xT": np.ascontiguousarray(x[:, e, :].T),
            "wi": np.ascontiguousarray(wi[e]),
            "wo": np.ascontiguousarray(wo[e]),
        }
        for e in range(E)
    ]
    try:
        res = run_bass_kernel_spmd(
            _NC, in_maps, core_ids=list(range(E)), **RUN_KWARGS
        )
    except Exception:
        res = run_bass_kernel_spmd(
            _NC, in_maps, core_ids=list(range(E)), **RUN_KWARGS
        )
    LAST_RESULT = res
    out = np.stack([res.results[e]["y"] for e in range(E)], axis=1)
    return np.ascontiguousarray(out.astype(np.float32, copy=False))



# revision 4
# speedup vs baseline: 1.0561x; 1.0561x over previous
"""Expert-parallel MoE MLP (BaseMLPExperts) for 8 TRN2 NeuronCores.

Reference computation (per expert e):
    y[:, e, :] = gelu_exact(x[:, e, :] @ wi[e]) @ wo[e]
with T=8192 tokens, E=8 experts, H=1024 hidden, I=4096 intermediate, fp32.

Sharding: expert-parallel — core e owns expert e (its x slice, wi[e], wo[e]).
No cross-core communication.

Per-core device kernel — fully fused, bf16 matmul inputs (PE runs bf16 at
the same 1 column/cycle rate as f32r, so this costs nothing in FLOP rate
but lets everything stay resident in SBUF; measured end-to-end rel-err
~3.4e-3 vs the 2e-2 gate):

  Both weights live in SBUF for the whole kernel (wi 64KB/partition +
  wo 64KB/partition as bf16). Per 512-token tile (16 tiles):
    GEMM1: for each of 32 i-blocks, 8 accumulating matmuls
           (wi block stationary, x tile moving) -> PSUM [128, 512];
           ACT engine applies exact-erf GELU on eviction, writing bf16
           h1 [128 (i), 32 (i-blk), 512 (t)] into SBUF (32KB/partition).
    GEMM2: for each of 4 128-token sub-blocks, 2 PSUM banks accumulate
           32 matmuls each (h1 block stationary, wo block moving);
           DVE evacuates y [128, 1024] f32 to SBUF, DMA to DRAM.
  h1 never touches DRAM; the PE sees one uninterrupted matmul stream
  (HAM stays warm, no phase boundaries, no DMA waits after priming).

Host side: transposes x slices to xT (H-major) and converts x/wi/wo to
bf16, runs the SPMD kernel on cores 0-7, stacks per-core y into [T, E, H].
"""

import ml_dtypes
import numpy as np

import concourse.bass as bass
import concourse.mybir as mybir
import concourse.tile as tile
from concourse import bacc
from concourse.bass_utils import run_bass_kernel_spmd

T, E, H, I = 8192, 8, 1024, 4096
P = 128
F32 = mybir.dt.float32
BF16 = mybir.dt.bfloat16

TT = 512             # token tile
NT = T // TT         # 16
HT = H // P          # 8 k-tiles for GEMM1
IT = I // P          # 32 i-tiles
TB = 128             # GEMM2 token sub-block
NB = TT // TB        # 4

# run_bass_kernel_spmd kwargs injected by test harness (e.g. trace=True)
RUN_KWARGS: dict = {}
LAST_RESULT = None

_NC = None


def _build():
    nc = bacc.Bacc("TRN2", target_bir_lowering=False, debug=False, num_devices=8)

    xT = nc.dram_tensor("xT", [H, T], BF16, kind="ExternalInput").ap()
    wi = nc.dram_tensor("wi", [H, I], BF16, kind="ExternalInput").ap()
    wo = nc.dram_tensor("wo", [I, H], BF16, kind="ExternalInput").ap()
    y = nc.dram_tensor("y", [T, H], F32, kind="ExternalOutput").ap()

    xT_r = xT.rearrange("(ho p) t -> p ho t", p=P)      # [128, 8, T]
    wi_r = wi.rearrange("(ho p) i -> p ho i", p=P)      # [128, 8, I]
    wo_r = wo.rearrange("(io p) h -> p io h", p=P)      # [128, 32, H]

    with tile.TileContext(nc) as tc:
        w_pool = tc.alloc_tile_pool(name="w_pool", bufs=1)
        wi_sb = w_pool.tile([P, HT, I], BF16, name="wi_sb")
        wo_sb = w_pool.tile([P, IT, H], BF16, name="wo_sb")

        with (
            tc.tile_pool(name="xt_pool", bufs=2) as xt_pool,
            tc.tile_pool(name="h1_pool", bufs=1) as h1_pool,
            tc.tile_pool(name="yo_pool", bufs=3) as yo_pool,
            tc.tile_pool(name="ps1_pool", bufs=4, space="PSUM") as ps1_pool,
            tc.tile_pool(name="ps2_pool", bufs=4, space="PSUM") as ps2_pool,
        ):
            def load_xt(tt, engs=(nc.sync, nc.scalar)):
                t0 = tt * TT
                xt = xt_pool.tile([P, HT, TT], BF16, name="xt", tag="xt")
                n = HT // len(engs)
                for g, eng in enumerate(engs):
                    eng.dma_start(
                        out=xt[:, n * g : n * (g + 1), :],
                        in_=xT_r[:, n * g : n * (g + 1), t0 : t0 + TT],
                    )
                return xt

            # Priming, in consumption order: xt(0) striped across all four
            # DMA-issue queues, then wi in 512-i-column chunks (i-major so
            # igroup i only waits for chunk i//4), then wo (first needed
            # ~55us in, fully streamed by ~30us).
            prime_engs = [nc.sync, nc.scalar, nc.gpsimd]
            xt_cur = load_xt(0, engs=(nc.sync, nc.scalar, nc.gpsimd, nc.sync))
            for q in range(8):
                prime_engs[q % 3].dma_start(
                    out=wi_sb[:, :, q * 512 : (q + 1) * 512],
                    in_=wi_r[:, :, q * 512 : (q + 1) * 512],
                )
            for q in range(8):
                prime_engs[q % 3].dma_start(
                    out=wo_sb[:, 4 * q : 4 * q + 4, :],
                    in_=wo_r[:, 4 * q : 4 * q + 4, :],
                )

            for tt in range(NT):
                # ---- GEMM1 + GELU: h1 = gelu(x @ wi), i on partitions ----
                h1 = h1_pool.tile([P, IT, TT], BF16, name="h1", tag="h1")
                for i in range(IT):
                    ps = ps1_pool.tile([P, TT], F32, name="ps1", tag="ps1")
                    for h in range(HT):
                        nc.tensor.matmul(
                            ps[:],
                            wi_sb[:, h, i * P : (i + 1) * P],
                            xt_cur[:, h, :],
                            start=(h == 0),
                            stop=(h == HT - 1),
                        )
                    nc.scalar.activation(
                        h1[:, i, :], ps[:], mybir.ActivationFunctionType.Gelu
                    )
                if tt + 1 < NT:
                    xt_nxt = load_xt(tt + 1)

                # ---- GEMM2: y = h1 @ wo ----
                for tb in range(NB):
                    t0 = tt * TT + tb * TB
                    pss = [
                        ps2_pool.tile([P, 512], F32, name="ps2", tag="ps2")
                        for _ in range(2)
                    ]
                    for i in range(IT):
                        for hh in range(2):
                            nc.tensor.matmul(
                                pss[hh][:],
                                h1[:, i, tb * TB : (tb + 1) * TB],
                                wo_sb[:, i, hh * 512 : (hh + 1) * 512],
                                start=(i == 0),
                                stop=(i == IT - 1),
                            )
                    yo = yo_pool.tile([P, H], F32, name="yo", tag="yo")
                    for hh in range(2):
                        nc.vector.tensor_copy(
                            yo[:, hh * 512 : (hh + 1) * 512], pss[hh][:]
                        )
                    nc.gpsimd.dma_start(out=y[t0 : t0 + TB, :], in_=yo[:])
                if tt + 1 < NT:
                    xt_cur = xt_nxt
        w_pool.release()

    nc.compile()
    return nc


def kernel(x: np.ndarray, wi: np.ndarray, wo: np.ndarray) -> np.ndarray:
    global _NC, LAST_RESULT
    x = np.asarray(x, dtype=np.float32)
    wi = np.asarray(wi, dtype=np.float32)
    wo = np.asarray(wo, dtype=np.float32)
    assert x.shape == (T, E, H) and wi.shape == (E, H, I) and wo.shape == (E, I, H)

    if _NC is None:
        _NC = _build()

    bf = ml_dtypes.bfloat16
    in_maps = [
        {
            "xT": np.ascontiguousarray(x[:, e, :].T).astype(bf),
            "wi": wi[e].astype(bf),
            "wo": wo[e].astype(bf),
        }
        for e in range(E)
    ]
    try:
        res = run_bass_kernel_spmd(
            _NC, in_maps, core_ids=list(range(E)), **RUN_KWARGS
        )
    except Exception:
        res = run_bass_kernel_spmd(
            _NC, in_maps, core_ids=list(range(E)), **RUN_KWARGS
        )
    LAST_RESULT = res
    out = np.stack([res.results[e]["y"] for e in range(E)], axis=1)
    return np.ascontiguousarray(out.astype(np.float32, copy=False))
